# revision 63
# baseline (speedup 1.0000x reference)
"""Trainium2 Bass kernel for a 3-layer edge-featured GAT over 256 dense 84-node graphs.

Contract: kernel(**inputs) takes the FULL unsharded inputs (as produced by the
problem's setup_inputs) and returns the FULL [256, 1] float32 output.

Strategy (data parallel over graphs, 32 graphs/core on 8 cores):
  Each graph is dense (all ordered pairs + self loops), so message passing
  collapses to dense per-graph [84, 84] attention planes. Host-side we build
  per-layer source-major logit planes E_l[s, (g, d)] (edge MLP folded to a
  scalar per edge, PyG mean self-loop attr on the diagonal) and augmented
  projection weights whose columns produce, per node: a copy of the previous
  layer's softmax denominator, the projected features h~, and the a_src /
  a_dst attention scalars.

  Softmax normalization is deferred: each layer carries the unnormalized
  aggregate U plus the denominator row den = sum_s ex in feature-major form
  ([den; relu(U)]), and the division by den folds into the node-major
  transpose copy as a per-partition reciprocal broadcast on the DVE. The
  per-layer flow is chunked by 4 graphs (336 columns = one PSUM bank):

    proj chunk -> PSUM -> SBUF -> per-graph PE transpose (node-major
    [den | h~ | a_src | a_dst]) -> DVE reciprocal of den column -> ONE DVE
    tensor_tensor normalizes h~ + a_src + a_dst into the hnode tile -> one
    small PE transpose turns the attention columns back into feature-major
    rows, landing (one cross-partition copy) in the logits stationary ->
    one 88-row logits matmul (E plane + graph masks x a_src) plus 4 rank-1
    a_dst matmuls accumulate the logit plane -> exp(lrelu) as one DVE
    scalar_tensor_tensor + one ScalarE Exp -> per-graph aggregation with
    stationary [1 | h~] produces feature-major [den; U] directly.

  Everything (E planes, masks, weights) is SBUF-resident after the prologue:
  the steady state issues ZERO DMAs (the baseline's ~100 dma_starts at a
  flat ~600ns sequencer cost each were the dominant bottleneck). All matmul
  operands are fp16 (PSUM accumulation fp32).
"""

import sys

for _p in ("/opt/trn_rl_repo",):
    if _p not in sys.path:
        sys.path.append(_p)

import numpy as np

from contextlib import ExitStack

from concourse import bacc, bass, mybir, tile
from concourse.bass_types import AP
from concourse.bass_utils import run_bass_kernel_spmd

F32 = mybir.dt.float32
F16 = mybir.dt.float16
AF = mybir.ActivationFunctionType
ALU = mybir.AluOpType

NPG = 84            # nodes per graph
B = 256             # graphs
HID = 64
DEPTH = 3
NEG_SLOPE = 0.2
NC_CORES = 8
GPC = B // NC_CORES     # 32 graphs per core
NB = GPC * NPG          # 2688 nodes per core
GCH = 4                 # graphs per chunk
CH = GCH * NPG          # 336 free-dim chunk (one PSUM bank in fp32)
NCH = GPC // GCH        # 8 chunks

# pTr / CW / ptT / hnode feature order: [den | h~(64) | a_src | a_dst]
NPROJ = 67              # projection rows, layers 0/1
HB = HID + 1            # hnode block width: [1 | h~(64)]
NPROJ2 = 4              # layer 2: [den | v | a_src | a_dst]
# Logits matmul, processed in half-layers of HGR = 16 graphs: the stationary
# lhsA = [a_src(0..15) | identity(16..99) | ones(100..115)] is the SAME
# [116, 84] tile for all 4 chunks of a half, and the moving rows are
# EM = [mask16(0..15) | E(16..99) | blockdiag a_dst(100..115)]. mask16 row
# (4c''+g) selects columns of graph g in chunks congruent to c'' (mod 4), so
# only the current half's a_src/a_dst rows act on its columns. The runtime
# rows come from one PE transpose of the normalized attention columns per
# half: a_src via a base-0 engine copy, a_dst via one SBUF->SBUF DMA that
# scatters the rows block-diagonally into EM.
HGR = 4 * GCH           # graphs per half-layer
NLR = 2 * HGR + NPG     # 116 logits matmul rows


def _host_preprocess(inputs):
    x = np.ascontiguousarray(np.asarray(inputs['x'], np.float32))
    ei = np.asarray(inputs['edge_index'])
    ea = np.asarray(inputs['edge_attr'], np.float32)
    W0 = np.asarray(inputs['W0'], np.float32)
    Ws = np.asarray(inputs['Ws'], np.float32)
    asl = np.asarray(inputs['att_src_all'], np.float32)
    adl = np.asarray(inputs['att_dst_all'], np.float32)
    Wel = np.asarray(inputs['W_edge_all'], np.float32)
    ael = np.asarray(inputs['att_edge_all'], np.float32)
    bl = np.asarray(inputs['bias_all'], np.float32)
    linW = np.asarray(inputs['lin_W'], np.float32)
    linb = np.asarray(inputs['lin_b'], np.float32)

    src, dst = np.asarray(ei[0]), np.asarray(ei[1])
    g = src // NPG
    assert np.all(dst // NPG == g), "edges cross graph boundaries"
    sl, dl = src % NPG, dst % NPG

    dense = np.zeros((B, NPG, NPG, 2), np.float32)
    dense[g, sl, dl] = ea
    cnt = np.zeros((B, NPG), np.float32)
    np.add.at(cnt, (g, dl), 1.0)
    colsum = dense.sum(axis=1)
    loop_attr = colsum / np.maximum(cnt, 1.0)[..., None]
    di = np.arange(NPG)
    dense[:, di, di, :] = loop_attr

    Es = []
    for l in range(DEPTH):
        w2 = Wel[l] @ ael[l]
        Es.append(np.ascontiguousarray(dense @ w2, dtype=np.float16))  # [B, s, d]

    # half-layer graph masks: mask16[r, j] = 1 iff (j // NPG) % HGR == r
    mask16 = np.zeros((HGR, NB), np.float16)
    jj = np.arange(NB)
    mask16[(jj // NPG) % HGR, jj] = 1.0

    # augmented projection weights; moving rows are [den; h-features]
    W_all = [W0, Ws[0], Ws[1]]
    cwpack = np.zeros((HID + 1, 2 * NPROJ + NPROJ2), np.float16)
    for l in range(2):
        K = W_all[l].shape[0]          # 1 (layer 0) or 64
        A = np.zeros((1 + K, NPROJ), np.float32)
        A[0, 0] = 1.0                  # den passthrough
        A[1:, 1:1 + HID] = W_all[l]
        A[1:, 1 + HID] = W_all[l] @ asl[l]
        A[1:, 2 + HID] = W_all[l] @ adl[l]
        cwpack[0:1 + K, l * NPROJ:(l + 1) * NPROJ] = A
    A2 = np.zeros((HID + 1, NPROJ2), np.float32)
    A2[0, 0] = 1.0
    A2[1:, 1] = (W_all[2] @ linW)[:, 0]
    A2[1:, 2] = W_all[2] @ asl[2]
    A2[1:, 3] = W_all[2] @ adl[2]
    cwpack[:, 2 * NPROJ:] = A2

    # logits stationary const part (one 84-col tile, shared by all chunks):
    # [a_src placeholder(0..15) | identity(16..99) | ones(100..115)]
    lhsC = np.zeros((NLR, NPG), np.float16)
    lhsC[HGR:HGR + NPG, :] = np.eye(NPG, dtype=np.float16)
    lhsC[HGR + NPG:, :] = 1.0

    ident = np.eye(NPG, dtype=np.float16)

    # sel16 block k: row k all-ones (picks a_dst row 4c''+g out of adS)
    sel16 = np.zeros((HGR, HGR * NPG), np.float16)
    for k in range(HGR):
        sel16[k, k * NPG:(k + 1) * NPG] = 1.0

    x_aug = np.ones((2, B * NPG), np.float16)   # rows [den=1; x]
    x_aug[1] = x[:, 0].astype(np.float16)

    tail_bias = float(NPG * float(bl[DEPTH - 1] @ linW[:, 0]) + float(linb[0]))
    assert not np.any(bl[:DEPTH - 1]), "nonzero conv bias not supported"

    return dict(x_aug=x_aug, Es=Es, mask16=mask16, cwpack=cwpack, lhsC=lhsC,
                ident=ident, sel16=sel16, tail_bias=tail_bias)


def _bcast_inner(ap, n):
    """View `ap` with an extra innermost stride-0 axis of length n."""
    return AP(ap.tensor, ap.offset, list(ap.ap) + [[0, n]])


def _build_program(tail_bias):
    nc = bacc.Bacc("TRN2", target_bir_lowering=False, debug=False)

    xh_d = nc.dram_tensor("xh", [2, NB], F16, kind="ExternalInput").ap()
    EM_d = [nc.dram_tensor(f"EM{l}", [NLR, NB], F16, kind="ExternalInput").ap()
            for l in range(DEPTH)]
    cw_d = nc.dram_tensor("cw", [HID + 1, 2 * NPROJ + NPROJ2], F16,
                          kind="ExternalInput").ap()
    lhsC_d = nc.dram_tensor("lhsC", [NLR, NPG], F16,
                            kind="ExternalInput").ap()
    ident_d = nc.dram_tensor("ident", [NPG, NPG], F16, kind="ExternalInput").ap()
    sel16_d = nc.dram_tensor("sel16", [HGR, HGR * NPG], F16,
                             kind="ExternalInput").ap()
    out_d = nc.dram_tensor("out", [GPC], F32, kind="ExternalOutput").ap()

    with tile.TileContext(nc) as tc, ExitStack() as ctx:
        cpool = ctx.enter_context(tc.tile_pool(name="const", bufs=1))
        ppool = ctx.enter_context(tc.tile_pool(name="proj", bufs=2))
        hpool = ctx.enter_context(tc.tile_pool(name="h", bufs=2))
        hnpool = ctx.enter_context(tc.tile_pool(name="hnode", bufs=2))
        expool = ctx.enter_context(tc.tile_pool(name="ex", bufs=2))
        lrpool = ctx.enter_context(tc.tile_pool(name="lr", bufs=4))
        smpool = ctx.enter_context(tc.tile_pool(name="small", bufs=4))

        aapool = ctx.enter_context(tc.tile_pool(name="aa", bufs=2))

        pp = ctx.enter_context(tc.tile_pool(name="pp", bufs=2, space="PSUM"))
        pt = ctx.enter_context(tc.tile_pool(name="pt", bufs=2, space="PSUM"))
        pl = ctx.enter_context(tc.tile_pool(name="pl", bufs=2, space="PSUM"))
        pa = ctx.enter_context(tc.tile_pool(name="pa", bufs=2, space="PSUM"))

        # ---- prologue: everything loads once, spread across 3 DMA queues,
        # ordered so the first chunk's dependencies land first ----
        xh_sb = cpool.tile([2, NB], F16, tag="xh")
        cw_sb = cpool.tile([HID + 1, 2 * NPROJ + NPROJ2], F16, tag="cw")
        ident_sb = cpool.tile([NPG, NPG], F16, tag="ident")
        lhsA0 = cpool.tile([NLR, NPG], F16, tag="lhsA0")
        lhsA1 = cpool.tile([NLR, NPG], F16, tag="lhsA1")
        lhsA_t = (lhsA0, lhsA1)
        EM_sb = []
        for l in range(DEPTH):
            em_t = cpool.tile([NLR, NB], F16, tag=f"EM{l}")
            EM_sb.append(em_t)
        nc.sync.dma_start(xh_sb[:], xh_d[:])
        nc.sync.dma_start(ident_sb[:], ident_d[:])
        nc.sync.dma_start(EM_sb[0][:], EM_d[0][:])
        nc.scalar.dma_start(cw_sb[:], cw_d[:])
        sel16_sb = cpool.tile([HGR, HGR * NPG], F16, tag="sel16")
        nc.scalar.dma_start(sel16_sb[:], sel16_d[:])
        nc.scalar.dma_start(EM_sb[1][:], EM_d[1][:])
        nc.gpsimd.dma_start(lhsA0[:], lhsC_d[:])
        nc.gpsimd.dma_start(lhsA1[:], lhsC_d[:])
        nc.gpsimd.dma_start(EM_sb[2][:], EM_d[2][:])
        ones84 = cpool.tile([NPG, 1], F16, tag="ones84")
        nc.vector.memset(ones84[:], 1.0)
        vo = cpool.tile([NPG, 2 * GPC], F16, tag="vo")
        nc.vector.memset(vo[:], 1.0)

        def chunk_cols(c):
            return slice(c * CH, (c + 1) * CH)

        def attention(l, c, pTr, nproj, lrb, recip, aa_all, hnode=None,
                      vo=None):
            """Per-chunk: node-major transpose + normalize into hnode/aa_all.
            pTr rows: [den | payload... | a_src | a_dst]."""
            bs = nproj + (nproj % 2)   # PSUM writes need 4-byte alignment
            ptT = pt.tile([NPG, 448], F16, tag="pt")
            for j in range(GCH):
                gg = c * GCH + j
                nc.tensor.transpose(ptT[0:NPG, j * bs:j * bs + nproj],
                                    pTr[:, gg * NPG:(gg + 1) * NPG],
                                    ident_sb[0:nproj, 0:nproj])
            ptT3 = (ptT[0:NPG, 0:GCH * bs]
                    .rearrange("p (g f) -> p g f", f=bs)[:, :, 0:nproj])
            rc = recip[:, c * GCH:(c + 1) * GCH]
            nc.vector.reciprocal(rc, ptT[0:NPG, 0:GCH * bs:bs])
            # normalized [a_src | a_dst] node-major into the half's aa_all
            # columns: col 32h + 16q + 4c'' + g
            aa3 = AP(aa_all.tensor,
                     aa_all.offset + 48 * (c // 4) + 4 * (c % 4),
                     [list(aa_all.ap[0]), [1, GCH], [2 * HGR, 2]])
            src3 = AP(ptT.tensor, ptT.offset + (nproj - 2),
                      [list(ptT.ap[0]), [bs, GCH], [1, 2]])
            nc.vector.tensor_tensor(
                aa3, src3,
                AP(rc.tensor, rc.offset, [list(rc.ap[0]), [1, GCH], [0, 2]]),
                ALU.mult)
            if hnode is not None:
                # normalized h~ in one op
                hn3 = (hnode[:, c * GCH * HB:(c + 1) * GCH * HB]
                       .rearrange("p (g f) -> p g f", f=HB)[:, :, 1:1 + HID])
                nc.vector.tensor_tensor(hn3, ptT3[:, :, 1:1 + HID],
                                        _bcast_inner(rc, HID), ALU.mult)
            else:
                # layer 2: normalized [v] into vo
                vo3 = (vo[:, 2 * c * GCH:2 * (c + 1) * GCH]
                       .rearrange("p (g q) -> p g q", q=2)[:, :, 0:1])
                nc.vector.tensor_tensor(vo3, ptT3[:, :, 1:2],
                                        _bcast_inner(rc, 1), ALU.mult)
            return ptT

        def att_rows_half(l, h, aa_all, ptT):
            """Per half-layer: one PE transpose turns the 32 normalized
            attention columns into feature-major rows; a_src rows go to the
            half's stationary by engine copy, a_dst rows scatter into EM's
            block-diagonal moving rows via one SBUF->SBUF DMA. The transpose
            lands in spare columns of the last chunk's ptT psum tile."""
            aaT = ptT[0:3 * HGR, 360:360 + NPG]
            nc.tensor.transpose(aaT, aa_all[:, 48 * h:48 * (h + 1)],
                                ident_sb[:])
            nc.scalar.copy(lhsA_t[h][0:HGR, :], ptT[0:HGR, 360:360 + NPG])
            adS = smpool.tile([HGR, NPG], F16, tag="adS")
            nc.vector.tensor_copy(adS[:], ptT[2 * HGR:3 * HGR,
                                              360:360 + NPG])
            return adS

        def logits_lrelu(l, c, lrb, adS):
            """Logit plane: E + mask*a_src matmul plus 4 selector matmuls
            for a_dst, then lrelu into lrb[:, chunk]."""
            cs = chunk_cols(c)
            ps_l = pl.tile([NPG, CH], F32, tag="pl")
            nc.tensor.matmul(ps_l[:], lhsA_t[c // 4][:], EM_sb[l][:, cs],
                             start=True, stop=False, skip_group_check=True)
            for j in range(GCH):
                k = 4 * (c % 4) + j
                nc.tensor.matmul(ps_l[:, j * NPG:(j + 1) * NPG],
                                 sel16_sb[:, k * NPG:(k + 1) * NPG], adS[:],
                                 start=False, stop=(j == GCH - 1),
                                 skip_group_check=True)
            # lrelu(x) = max(0.2x, x); a DVE op may read at most one PSUM
            # operand, so scale on ScalarE first, then max against PSUM
            lt = lrpool.tile([NPG, CH], F16, tag="lt")
            nc.scalar.mul(lt[:], ps_l[:], NEG_SLOPE)
            nc.vector.tensor_tensor(lrb[:, cs], lt[:], ps_l[:], ALU.max)

        def half_tail(l, h, adS, lrb, ex, hnode, hT_next):
            """Logits + exp + aggregation + relu for chunks of half h."""
            for cc in range(4 * h, 4 * h + 4):
                logits_lrelu(l, cc, lrb, adS)
                if cc % 2 == 1:
                    ps2 = slice((cc - 1) * CH, (cc + 1) * CH)
                    nc.scalar.activation(ex[:, ps2], lrb[:, ps2], AF.Exp)
                    for c2 in (cc - 1, cc):
                        # aggregation: stationary [1 | h~] -> [den; U]
                        ps_a = pa.tile([HID + 1, CH], F32, tag="pa")
                        for j in range(GCH):
                            gg = c2 * GCH + j
                            nc.tensor.matmul(
                                ps_a[:, j * NPG:(j + 1) * NPG],
                                hnode[:, gg * HB:gg * HB + HID + 1],
                                ex[:, gg * NPG:(gg + 1) * NPG],
                                start=True, stop=True)
                        c2s = chunk_cols(c2)
                        if c2 % 2 == 0:
                            nc.scalar.activation(hT_next[:, c2s], ps_a[:],
                                                 AF.Relu)
                        else:
                            nc.vector.tensor_scalar_max(hT_next[:, c2s],
                                                        ps_a[:], 0.0)

        def layer(l, hT_in, K):
            """Layers 0/1. hT_in: [K, NB], rows [den; h]. Returns hT [65, NB]."""
            cw_l = cw_sb[0:K, l * NPROJ:(l + 1) * NPROJ]
            pTr = ppool.tile([NPROJ, NB], F16, tag="pTr")
            lrb = ppool.tile([NPG, NB], F16, tag="lrb")
            ex = expool.tile([NPG, NB], F16, tag="ex")
            aa_all = aapool.tile([NPG, 96], F16, tag="aa_all")
            nc.gpsimd.memset(aa_all[:], 0.0)
            hnode = hnpool.tile([NPG, GPC * HB], F16, tag="hnode")
            nc.vector.memset(hnode[:, 0:GPC * HB:HB], 1.0)
            recip = smpool.tile([NPG, GPC], F32, tag="recip")
            hT_next = hpool.tile([HID + 1, NB], F16, tag="hT")
            for c in range(NCH):
                cs = chunk_cols(c)
                ps_p = pp.tile([NPROJ, CH], F32, tag="pp")
                nc.tensor.matmul(ps_p[:], cw_l, hT_in[:, cs],
                                 start=True, stop=True)
                if c % 2 == 0:
                    nc.vector.tensor_copy(pTr[:, cs], ps_p[:])
                else:
                    nc.scalar.copy(pTr[:, cs], ps_p[:])
                ptT = attention(l, c, pTr, NPROJ, lrb, recip, aa_all,
                                hnode=hnode)
                if c == 3:
                    adS0 = att_rows_half(l, 0, aa_all, ptT)
                elif c == 7:
                    adS1 = att_rows_half(l, 1, aa_all, ptT)
                    half_tail(l, 0, adS0, lrb, ex, hnode, hT_next)
                    half_tail(l, 1, adS1, lrb, ex, hnode, hT_next)
            return hT_next

        hT = layer(0, xh_sb, 2)
        hT = layer(1, hT, HID + 1)

        # ---- layer 2: attention + readout only ----
        cw_l = cw_sb[0:HID + 1, 2 * NPROJ:2 * NPROJ + NPROJ2]
        pT2 = ppool.tile([NPROJ2, NB], F16, tag="pTr")
        lrb = ppool.tile([NPG, NB], F16, tag="lrb")
        ex = expool.tile([NPG, NB], F16, tag="ex")
        aa_all = aapool.tile([NPG, 96], F16, tag="aa_all")
        nc.gpsimd.memset(aa_all[:], 0.0)
        recip = smpool.tile([NPG, GPC], F32, tag="recip")
        pq = pa.tile([NPG, 2 * GPC], F32, tag="pa")

        def half_tail2(h, adS):
            for cc in range(4 * h, 4 * h + 4):
                logits_lrelu(2, cc, lrb, adS)
                if cc % 2 == 1:
                    ps2 = slice((cc - 1) * CH, (cc + 1) * CH)
                    nc.scalar.activation(ex[:, ps2], lrb[:, ps2], AF.Exp)
                    # readout aggregation: stationary ex_g, moving [v | 1]
                    for gg in range((cc - 1) * GCH, (cc + 1) * GCH):
                        nc.tensor.matmul(pq[:, 2 * gg:2 * gg + 2],
                                         ex[:, gg * NPG:(gg + 1) * NPG],
                                         vo[:, 2 * gg:2 * gg + 2],
                                         start=True, stop=True)

        for c in range(NCH):
            cs = chunk_cols(c)
            ps_p = pp.tile([NPROJ2, CH], F32, tag="pp")
            nc.tensor.matmul(ps_p[:], cw_l, hT[:, cs], start=True, stop=True)
            if c % 2 == 0:
                nc.vector.tensor_copy(pT2[:, cs], ps_p[:])
            else:
                nc.scalar.copy(pT2[:, cs], ps_p[:])
            ptT = attention(2, c, pT2, NPROJ2, lrb, recip, aa_all, vo=vo)
            if c == 3:
                adS0 = att_rows_half(2, 0, aa_all, ptT)
            elif c == 7:
                adS1 = att_rows_half(2, 1, aa_all, ptT)
                half_tail2(0, adS0)
                half_tail2(1, adS1)
        recip3 = smpool.tile([NPG, GPC], F32, tag="recip3")
        nc.vector.reciprocal(recip3[:], pq[:, 1::2])
        qsb = smpool.tile([NPG, GPC], F16, tag="qsb")
        nc.vector.tensor_mul(qsb[:], pq[:, 0::2], recip3[:])
        ps_z = pp.tile([1, GPC], F32, tag="pp")
        nc.tensor.matmul(ps_z[:], ones84[:], qsb[:], start=True, stop=True)
        zout = smpool.tile([1, GPC], F32, tag="zout")
        nc.scalar.activation(zout[:], ps_z[:], AF.Relu, bias=float(tail_bias))
        nc.sync.dma_start(out_d.rearrange("(o g) -> o g", o=1), zout[:])

    nc.compile()
    return nc


def _core_inputs(pre, c):
    m = {
        'xh': np.ascontiguousarray(pre['x_aug'][:, c * NB:(c + 1) * NB]),
        'cw': pre['cwpack'], 'lhsC': pre['lhsC'], 'ident': pre['ident'],
        'sel16': pre['sel16'],
    }
    for l in range(DEPTH):
        E = np.ascontiguousarray(
            np.transpose(pre['Es'][l][c * GPC:(c + 1) * GPC], (1, 0, 2))
            .reshape(NPG, NB))
        m[f'EM{l}'] = np.ascontiguousarray(np.vstack(
            [pre['mask16'], E, np.zeros((HGR, NB), np.float16)]))
    return m


def kernel(**inputs):
    pre = _host_preprocess(inputs)
    nc = _build_program(pre['tail_bias'])
    in_maps = [_core_inputs(pre, c) for c in range(NC_CORES)]
    res = run_bass_kernel_spmd(nc, in_maps, list(range(NC_CORES)))
    out = np.concatenate([np.asarray(res.results[c]['out'])
                          for c in range(NC_CORES)])
    return out.reshape(B, 1).astype(np.float32)
